# revision 12
# baseline (speedup 1.0000x reference)
"""Multi-head attention TRN2 kernel, head-parallel over 8 NeuronCores.

Problem shape: B=2, S=2048, d_model=512, n_heads=8, head_dim=512
(the projections are Linear(512, 512*8), so each head has dim 512).

Sharding: core h computes head h for both batches (column-parallel
Wq/Wk/Wv, row-parallel Wo).  Each core returns its partial output
(O_h @ Wo_h) of shape [B*S, 512]; the host sums the 8 partials and adds
the bias constant (bv @ Wo + bo), which passes through attention
linearly because softmax rows sum to 1.

Device layout avoids all on-device transposes:
  - Host passes Q/K/V pre-transposed as [512, B*S] bf16.
  - q,k are produced transposed ([head_dim, S]); v natural ([S, head_dim]).
  - Scores are computed transposed (P^T tiles [Sk, Sq]); exp on ACT
    (no max subtraction: |scores| <= ~2.5 for this problem's scale).
  - PV matmuls with v as stationary give O^T directly; softmax
    denominators come from a ones-vector matmul over the same P^T tiles;
    normalization is a DVE multiply against a partition-broadcast
    reciprocal row. O^T blocks are then the stationary operand for the
    Wo matmul.
"""

import math

import numpy as np
import ml_dtypes

B = 2
S = 2048
D = 512          # d_model == head_dim
H = 8
N_CORES = 8
BS = B * S       # 4096
NT = D // 128    # 4 contraction tiles of 128
SQC = 512        # query-chunk (matmul moving free dim)
NSQ = S // SQC   # 4 chunks per batch
NKT = S // 128   # 16 key tiles per batch
SCALE = 1.0 / math.sqrt(float(D))

_compiled = None


def _body(nc, mybir, pools, aps):
    f32 = mybir.dt.float32
    bf16 = mybir.dt.bfloat16
    Id = mybir.ActivationFunctionType.Identity
    Exp = mybir.ActivationFunctionType.Exp
    (kvpool, instream, qinp, qpool, ptp, otp, rpool, outp,
     ps_proj, ps_s, ps_o) = pools
    (qt_d, kt_d, vt_d, out_d,
     wq_sb, wk_sb, wv_sb, wo_sb, bq_sb, bk_sb, ones_sb, ones_row) = aps

    for b in range(B):
        tok0 = S * b

        # --- k^T for this batch: kt_b[p, t, s] = k_h[128t+p, s] ---
        kin = []
        for t in range(NT):
            ki = instream.tile([128, S], bf16, tag="kin")
            nc.sync.dma_start(
                ki[:, :], kt_d[128 * t:128 * (t + 1), tok0:tok0 + S])
            kin.append(ki)
        kt_b = kvpool.tile([128, NT, S], bf16, tag="ktb")
        for m in range(NT):
            for c4 in range(NSQ):
                ps = ps_proj.tile([128, SQC], f32, tag="psp")
                for t in range(NT):
                    nc.tensor.matmul(
                        ps[:, :],
                        wk_sb[:, t, 128 * m:128 * (m + 1)],
                        kin[t][:, SQC * c4:SQC * (c4 + 1)],
                        start=(t == 0), stop=(t == NT - 1))
                nc.scalar.activation(
                    kt_b[:, m, SQC * c4:SQC * (c4 + 1)], ps[:, :], Id,
                    bias=bk_sb[:, m:m + 1])

        # --- v natural for this batch: v_b[p, j, e] = v_h[128j+p, e] ---
        vin = []
        for t in range(NT):
            vi = instream.tile([128, S], bf16, tag="vin")
            nc.sync.dma_start(
                vi[:, :], vt_d[128 * t:128 * (t + 1), tok0:tok0 + S])
            vin.append(vi)
        v_b = kvpool.tile([128, NKT, D], bf16, tag="vb")
        for j in range(NKT):
            ps = ps_proj.tile([128, D], f32, tag="psp")
            for t in range(NT):
                nc.tensor.matmul(
                    ps[:, :],
                    vin[t][:, 128 * j:128 * (j + 1)],
                    wv_sb[:, t, :],
                    start=(t == 0), stop=(t == NT - 1))
            nc.scalar.activation(v_b[:, j, :], ps[:, :], Id)

        for c in range(NSQ):
            q0 = tok0 + SQC * c

            # --- q^T chunk: qt_c[p, m, s] = q_h[128m+p, q0+s] ---
            qin = []
            for t in range(NT):
                qi = qinp.tile([128, SQC], bf16, tag="qin")
                nc.sync.dma_start(
                    qi[:, :], qt_d[128 * t:128 * (t + 1), q0:q0 + SQC])
                qin.append(qi)
            qt_c = qpool.tile([128, NT, SQC], bf16, tag="qtc")
            for m in range(NT):
                ps = ps_proj.tile([128, SQC], f32, tag="psp")
                for t in range(NT):
                    nc.tensor.matmul(
                        ps[:, :],
                        wq_sb[:, t, 128 * m:128 * (m + 1)],
                        qin[t][:, :],
                        start=(t == 0), stop=(t == NT - 1))
                nc.scalar.activation(
                    qt_c[:, m, :], ps[:, :], Id, bias=bq_sb[:, m:m + 1])

            # --- P^T tiles: pts[j][p, s] = exp(scores[q0+s, 128j+p]) ---
            pts = []
            for j in range(NKT):
                ps = ps_s.tile([128, SQC], f32, tag="pss")
                for t in range(NT):
                    nc.tensor.matmul(
                        ps[:, :],
                        kt_b[:, t, 128 * j:128 * (j + 1)],
                        qt_c[:, t, :],
                        start=(t == 0), stop=(t == NT - 1))
                pt_j = ptp.tile([128, SQC], bf16, tag="ptj")
                nc.scalar.activation(pt_j[:, :], ps[:, :], Exp, scale=SCALE)
                pts.append(pt_j)

            # --- softmax denominators, then O^T = v^T P^T ---
            pd = ps_o.tile([1, SQC], f32, tag="pso")
            for j in range(NKT):
                nc.tensor.matmul(pd[:, :], ones_sb[:, :], pts[j][:, :],
                                 start=(j == 0), stop=(j == NKT - 1))
            rec = rpool.tile([1, SQC], f32, tag="rec")
            nc.vector.reciprocal(rec[:, :], pd[:, :])
            # broadcast rec across partitions with a K=1 outer-product
            # matmul: [1,128] ones^T @ [1,512] rec -> psum [128,512],
            # then bounce to SBUF (DVE can read only one PSUM input)
            rec_ps = ps_s.tile([128, SQC], f32, tag="pss")
            nc.tensor.matmul(rec_ps[:, :], ones_row[:, :], rec[:, :],
                             start=True, stop=True)
            rec_full = rpool.tile([128, SQC], f32, tag="recb")
            nc.scalar.copy(rec_full[:, :], rec_ps[:, :])

            ot_c = otp.tile([128, NT, SQC], bf16, tag="otc")
            for m in range(NT):
                po = ps_o.tile([128, SQC], f32, tag="pso")
                for j in range(NKT):
                    nc.tensor.matmul(
                        po[:, :],
                        v_b[:, j, 128 * m:128 * (m + 1)],
                        pts[j][:, :],
                        start=(j == 0), stop=(j == NKT - 1))
                nc.vector.tensor_mul(
                    ot_c[:, m, :], po[:, :], rec_full[:, :])

            # --- partial out chunk: O @ Wo_h ---
            for jq in range(NSQ):
                pw = ps_proj.tile([128, D], f32, tag="psp")
                for m in range(NT):
                    nc.tensor.matmul(
                        pw[:, :],
                        ot_c[:, m, 128 * jq:128 * (jq + 1)],
                        wo_sb[:, m, :],
                        start=(m == 0), stop=(m == NT - 1))
                osb = outp.tile([128, D], f32, tag="osb")
                nc.scalar.copy(osb[:, :], pw[:, :])
                r0 = q0 + 128 * jq
                nc.sync.dma_start(out_d[r0:r0 + 128, :], osb[:, :])


def _build(loop_k=1):
    """Build the bass program. loop_k>1 wraps the whole body in a
    For_i hardware loop (used only for differential timing in test.py;
    the graded path uses loop_k=1)."""
    import contextlib

    import concourse.tile as tile
    from concourse import bacc, mybir

    f32 = mybir.dt.float32
    bf16 = mybir.dt.bfloat16

    nc = bacc.Bacc("TRN2", target_bir_lowering=False, debug=False,
                   num_devices=N_CORES)

    qt_d = nc.dram_tensor("qt", [D, BS], bf16, kind="ExternalInput").ap()
    kt_d = nc.dram_tensor("kt", [D, BS], bf16, kind="ExternalInput").ap()
    vt_d = nc.dram_tensor("vt", [D, BS], bf16, kind="ExternalInput").ap()
    wq_d = nc.dram_tensor("wq", [D, D], bf16, kind="ExternalInput").ap()
    wk_d = nc.dram_tensor("wk", [D, D], bf16, kind="ExternalInput").ap()
    wv_d = nc.dram_tensor("wv", [D, D], bf16, kind="ExternalInput").ap()
    wo_d = nc.dram_tensor("wo", [D, D], bf16, kind="ExternalInput").ap()
    bq_d = nc.dram_tensor("bq", [128, NT], f32, kind="ExternalInput").ap()
    bk_d = nc.dram_tensor("bk", [128, NT], f32, kind="ExternalInput").ap()
    out_d = nc.dram_tensor("out", [BS, D], f32, kind="ExternalOutput").ap()

    with tile.TileContext(nc) as tc:
        with (
            tc.tile_pool(name="weights", bufs=1) as wpool,
            tc.tile_pool(name="kv", bufs=2) as kvpool,
            tc.tile_pool(name="instream", bufs=4) as instream,
            tc.tile_pool(name="qin", bufs=8) as qinp,
            tc.tile_pool(name="q", bufs=2) as qpool,
            tc.tile_pool(name="pt", bufs=20) as ptp,
            tc.tile_pool(name="ot", bufs=2) as otp,
            tc.tile_pool(name="recip", bufs=2) as rpool,
            tc.tile_pool(name="outsb", bufs=8) as outp,
            tc.tile_pool(name="ps_proj", bufs=2, space="PSUM") as ps_proj,
            tc.tile_pool(name="ps_s", bufs=3, space="PSUM") as ps_s,
            tc.tile_pool(name="ps_o", bufs=2, space="PSUM") as ps_o,
        ):
            # --- weights / biases, resident for the whole kernel ---
            wq_sb = wpool.tile([128, NT, D], bf16, tag="wq")
            wk_sb = wpool.tile([128, NT, D], bf16, tag="wk")
            wv_sb = wpool.tile([128, NT, D], bf16, tag="wv")
            wo_sb = wpool.tile([128, NT, D], bf16, tag="wo")
            for t in range(NT):
                nc.sync.dma_start(wq_sb[:, t, :], wq_d[128 * t:128 * (t + 1), :])
                nc.sync.dma_start(wk_sb[:, t, :], wk_d[128 * t:128 * (t + 1), :])
                nc.sync.dma_start(wv_sb[:, t, :], wv_d[128 * t:128 * (t + 1), :])
                nc.sync.dma_start(wo_sb[:, t, :], wo_d[128 * t:128 * (t + 1), :])
            bq_sb = wpool.tile([128, NT], f32, tag="bq")
            bk_sb = wpool.tile([128, NT], f32, tag="bk")
            nc.sync.dma_start(bq_sb[:, :], bq_d[:, :])
            nc.sync.dma_start(bk_sb[:, :], bk_d[:, :])
            ones_sb = wpool.tile([128, 1], bf16, tag="ones")
            nc.vector.memset(ones_sb[:, :], 1.0)
            ones_row = wpool.tile([1, 128], f32, tag="onesr")
            nc.vector.memset(ones_row[:, :], 1.0)

            pools = (kvpool, instream, qinp, qpool, ptp, otp, rpool, outp,
                     ps_proj, ps_s, ps_o)
            aps = (qt_d, kt_d, vt_d, out_d,
                   wq_sb, wk_sb, wv_sb, wo_sb, bq_sb, bk_sb, ones_sb,
                   ones_row)
            loop_cm = (tc.For_i(0, loop_k, 1) if loop_k > 1
                       else contextlib.nullcontext())
            with loop_cm:
                _body(nc, mybir, pools, aps)

    nc.compile()
    return nc


def _get_compiled():
    global _compiled
    if _compiled is None:
        _compiled = _build()
    return _compiled


def _make_in_maps(Q, K, V, Wq, bq, Wk, bk, Wv, bv, Wo, bo):
    bf = ml_dtypes.bfloat16
    qt = np.ascontiguousarray(Q.reshape(BS, D).T).astype(bf)
    kt = np.ascontiguousarray(K.reshape(BS, D).T).astype(bf)
    vt = np.ascontiguousarray(V.reshape(BS, D).T).astype(bf)
    in_maps = []
    for h in range(N_CORES):
        sl = slice(D * h, D * (h + 1))
        in_maps.append({
            "qt": qt, "kt": kt, "vt": vt,
            "wq": np.ascontiguousarray(Wq[:, sl]).astype(bf),
            "wk": np.ascontiguousarray(Wk[:, sl]).astype(bf),
            "wv": np.ascontiguousarray(Wv[:, sl]).astype(bf),
            "wo": np.ascontiguousarray(Wo[sl, :]).astype(bf),
            "bq": np.ascontiguousarray(bq[sl].reshape(NT, 128).T).astype(np.float32),
            "bk": np.ascontiguousarray(bk[sl].reshape(NT, 128).T).astype(np.float32),
        })
    return in_maps


def kernel(Q, K, V, Wq, bq, Wk, bk, Wv, bv, Wo, bo, _trace=False, _trace_kwargs=None):
    from concourse.bass_utils import run_bass_kernel_spmd

    nc = _get_compiled()
    in_maps = _make_in_maps(Q, K, V, Wq, bq, Wk, bk, Wv, bv, Wo, bo)
    res = run_bass_kernel_spmd(nc, in_maps, core_ids=list(range(N_CORES)),
                               trace=_trace, **(_trace_kwargs or {}))
    kernel.last_results = res

    acc = np.zeros((BS, D), np.float64)
    for h in range(N_CORES):
        acc += res.results[h]["out"].astype(np.float64)
    const = bv.astype(np.float64) @ Wo.astype(np.float64) + bo.astype(np.float64)
    return (acc + const).astype(np.float32).reshape(B, S, D)


# revision 15
# speedup vs baseline: 1.4936x; 1.4936x over previous
"""Multi-head attention TRN2 kernel, head-parallel over 8 NeuronCores.

Problem shape: B=2, S=2048, d_model=512, n_heads=8, head_dim=512
(the projections are Linear(512, 512*8), so each head has dim 512).

Sharding: core h computes head h for both batches (column-parallel
Wq/Wk/Wv, row-parallel Wo).  Each core returns its partial output
(O_h @ Wo_h) of shape [B*S, 512]; the host sums the 8 partials and adds
the bias constant (bv @ Wo + bo), which passes through attention
linearly because softmax rows sum to 1.

Device layout avoids all on-device transposes:
  - Host passes Q/K/V pre-transposed as [512, B*S] bf16.
  - q,k are produced transposed ([head_dim, S]); v natural ([S, head_dim]).
  - Scores are computed transposed (P^T tiles [Sk, Sq]); exp on ACT
    (no max subtraction: |scores| <= ~2.5 for this problem's scale).
  - PV matmuls with v as stationary give O^T directly; softmax
    denominators come from a ones-vector matmul over the same P^T tiles;
    normalization is a DVE multiply against a partition-broadcast
    reciprocal row. O^T blocks are then the stationary operand for the
    Wo matmul.
"""

import math

import numpy as np
import ml_dtypes

B = 2
S = 2048
D = 512          # d_model == head_dim
H = 8
N_CORES = 8
BS = B * S       # 4096
NT = D // 128    # 4 contraction tiles of 128
SQC = 512        # query-chunk (matmul moving free dim)
NSQ = S // SQC   # 4 chunks per batch
NKT = S // 128   # 16 key tiles per batch
SCALE = 1.0 / math.sqrt(float(D))

_compiled = None


def _body(nc, mybir, pools, aps):
    f32 = mybir.dt.float32
    bf16 = mybir.dt.bfloat16
    Id = mybir.ActivationFunctionType.Identity
    Exp = mybir.ActivationFunctionType.Exp
    (kvpool, instream, qinp, qpool, ptp, otp, rpool, outp,
     ps_proj, ps_s, ps_o) = pools
    (qt_d, kt_d, vt_d, out_d,
     wq_sb, wk_sb, wv_sb, wo_sb, bq_sb, bk_sb, ones_sb, ones_row) = aps

    for b in range(B):
        tok0 = S * b

        # --- k^T for this batch: kt_b[p, t, s] = k_h[128t+p, s] ---
        kin = []
        for t in range(NT):
            ki = instream.tile([128, S], bf16, tag="kin")
            nc.sync.dma_start(
                ki[:, :], kt_d[128 * t:128 * (t + 1), tok0:tok0 + S])
            kin.append(ki)
        kt_b = kvpool.tile([128, NT, S], bf16, tag="ktb")
        for m in range(NT):
            for c4 in range(NSQ):
                ps = ps_proj.tile([128, SQC], f32, tag="psp")
                for t in range(NT):
                    nc.tensor.matmul(
                        ps[:, :],
                        wk_sb[:, t, 128 * m:128 * (m + 1)],
                        kin[t][:, SQC * c4:SQC * (c4 + 1)],
                        start=(t == 0), stop=(t == NT - 1))
                nc.scalar.activation(
                    kt_b[:, m, SQC * c4:SQC * (c4 + 1)], ps[:, :], Id,
                    bias=bk_sb[:, m:m + 1])

        # --- v natural for this batch: v_b[p, j, e] = v_h[128j+p, e] ---
        vin = []
        for t in range(NT):
            vi = instream.tile([128, S], bf16, tag="vin")
            nc.sync.dma_start(
                vi[:, :], vt_d[128 * t:128 * (t + 1), tok0:tok0 + S])
            vin.append(vi)
        v_b = kvpool.tile([128, NKT, D], bf16, tag="vb")
        for j in range(NKT):
            ps = ps_proj.tile([128, D], f32, tag="psp")
            for t in range(NT):
                nc.tensor.matmul(
                    ps[:, :],
                    vin[t][:, 128 * j:128 * (j + 1)],
                    wv_sb[:, t, :],
                    start=(t == 0), stop=(t == NT - 1))
            nc.scalar.activation(v_b[:, j, :], ps[:, :], Id)

        for c in range(NSQ):
            q0 = tok0 + SQC * c

            # --- q^T chunk: qt_c[p, m, s] = q_h[128m+p, q0+s] ---
            qin = []
            for t in range(NT):
                qi = qinp.tile([128, SQC], bf16, tag="qin")
                nc.sync.dma_start(
                    qi[:, :], qt_d[128 * t:128 * (t + 1), q0:q0 + SQC])
                qin.append(qi)
            qt_c = qpool.tile([128, NT, SQC], bf16, tag="qtc")
            for m in range(NT):
                ps = ps_proj.tile([128, SQC], f32, tag="psp")
                for t in range(NT):
                    nc.tensor.matmul(
                        ps[:, :],
                        wq_sb[:, t, 128 * m:128 * (m + 1)],
                        qin[t][:, :],
                        start=(t == 0), stop=(t == NT - 1))
                nc.scalar.activation(
                    qt_c[:, m, :], ps[:, :], Id, bias=bq_sb[:, m:m + 1])

            # --- P^T tiles: pts[j][p, s] = exp(scores[q0+s, 128j+p]) ---
            pts = []
            for j in range(NKT):
                ps = ps_s.tile([128, SQC], f32, tag="pss")
                for t in range(NT):
                    nc.tensor.matmul(
                        ps[:, :],
                        kt_b[:, t, 128 * j:128 * (j + 1)],
                        qt_c[:, t, :],
                        start=(t == 0), stop=(t == NT - 1))
                pt_j = ptp.tile([128, SQC], bf16, tag="ptj")
                nc.scalar.activation(pt_j[:, :], ps[:, :], Exp, scale=SCALE)
                pts.append(pt_j)

            # --- softmax denominators, then O^T = v^T P^T ---
            pd = ps_o.tile([1, SQC], f32, tag="pso")
            for j in range(NKT):
                nc.tensor.matmul(pd[:, :], ones_sb[:, :], pts[j][:, :],
                                 start=(j == 0), stop=(j == NKT - 1))
            rec = rpool.tile([1, SQC], f32, tag="rec")
            nc.vector.reciprocal(rec[:, :], pd[:, :])
            # broadcast rec across partitions with a K=1 outer-product
            # matmul: [1,128] ones^T @ [1,512] rec -> psum [128,512],
            # then bounce to SBUF (DVE can read only one PSUM input)
            rec_ps = ps_s.tile([128, SQC], f32, tag="pss")
            nc.tensor.matmul(rec_ps[:, :], ones_row[:, :], rec[:, :],
                             start=True, stop=True)
            rec_full = rpool.tile([128, SQC], f32, tag="recb")
            nc.scalar.copy(rec_full[:, :], rec_ps[:, :])

            ot_c = otp.tile([128, NT, SQC], bf16, tag="otc")
            for m in range(NT):
                po = ps_o.tile([128, SQC], f32, tag="pso")
                for j in range(NKT):
                    nc.tensor.matmul(
                        po[:, :],
                        v_b[:, j, 128 * m:128 * (m + 1)],
                        pts[j][:, :],
                        start=(j == 0), stop=(j == NKT - 1))
                nc.vector.tensor_mul(
                    ot_c[:, m, :], po[:, :], rec_full[:, :])

            # --- partial out chunk: O @ Wo_h ---
            for jq in range(NSQ):
                pw = ps_proj.tile([128, D], f32, tag="psp")
                for m in range(NT):
                    nc.tensor.matmul(
                        pw[:, :],
                        ot_c[:, m, 128 * jq:128 * (jq + 1)],
                        wo_sb[:, m, :],
                        start=(m == 0), stop=(m == NT - 1))
                osb = outp.tile([128, D], f32, tag="osb")
                nc.scalar.copy(osb[:, :], pw[:, :])
                r0 = q0 + 128 * jq
                nc.sync.dma_start(out_d[r0:r0 + 128, :], osb[:, :])


def _build(loop_k=1, bench=False):
    """Build the bass program. loop_k>1 wraps the whole body in a
    For_i hardware loop; bench=True demotes the real output to an
    internal DRAM buffer and emits a tiny dummy ExternalOutput (both
    used only for differential timing in test.py; the graded path uses
    loop_k=1, bench=False)."""
    import contextlib

    import concourse.tile as tile
    from concourse import bacc, mybir

    f32 = mybir.dt.float32
    bf16 = mybir.dt.bfloat16

    nc = bacc.Bacc("TRN2", target_bir_lowering=False, debug=False,
                   num_devices=N_CORES)

    qt_d = nc.dram_tensor("qt", [D, BS], bf16, kind="ExternalInput").ap()
    kt_d = nc.dram_tensor("kt", [D, BS], bf16, kind="ExternalInput").ap()
    vt_d = nc.dram_tensor("vt", [D, BS], bf16, kind="ExternalInput").ap()
    wq_d = nc.dram_tensor("wq", [D, D], bf16, kind="ExternalInput").ap()
    wk_d = nc.dram_tensor("wk", [D, D], bf16, kind="ExternalInput").ap()
    wv_d = nc.dram_tensor("wv", [D, D], bf16, kind="ExternalInput").ap()
    wo_d = nc.dram_tensor("wo", [D, D], bf16, kind="ExternalInput").ap()
    bq_d = nc.dram_tensor("bq", [128, NT], f32, kind="ExternalInput").ap()
    bk_d = nc.dram_tensor("bk", [128, NT], f32, kind="ExternalInput").ap()
    if bench:
        out_d = nc.dram_tensor("outbuf", [BS, D], f32).ap()
        dummy_d = nc.dram_tensor("out", [128, NT], f32,
                                 kind="ExternalOutput").ap()
    else:
        out_d = nc.dram_tensor("out", [BS, D], f32, kind="ExternalOutput").ap()
        dummy_d = None

    with tile.TileContext(nc) as tc:
        with (
            tc.tile_pool(name="weights", bufs=1) as wpool,
            tc.tile_pool(name="kv", bufs=2) as kvpool,
            tc.tile_pool(name="instream", bufs=4) as instream,
            tc.tile_pool(name="qin", bufs=8) as qinp,
            tc.tile_pool(name="q", bufs=2) as qpool,
            tc.tile_pool(name="pt", bufs=20) as ptp,
            tc.tile_pool(name="ot", bufs=2) as otp,
            tc.tile_pool(name="recip", bufs=2) as rpool,
            tc.tile_pool(name="outsb", bufs=8) as outp,
            tc.tile_pool(name="ps_proj", bufs=2, space="PSUM") as ps_proj,
            tc.tile_pool(name="ps_s", bufs=3, space="PSUM") as ps_s,
            tc.tile_pool(name="ps_o", bufs=2, space="PSUM") as ps_o,
        ):
            # --- weights / biases, resident for the whole kernel ---
            wq_sb = wpool.tile([128, NT, D], bf16, tag="wq")
            wk_sb = wpool.tile([128, NT, D], bf16, tag="wk")
            wv_sb = wpool.tile([128, NT, D], bf16, tag="wv")
            wo_sb = wpool.tile([128, NT, D], bf16, tag="wo")
            for t in range(NT):
                nc.sync.dma_start(wq_sb[:, t, :], wq_d[128 * t:128 * (t + 1), :])
                nc.sync.dma_start(wk_sb[:, t, :], wk_d[128 * t:128 * (t + 1), :])
                nc.sync.dma_start(wv_sb[:, t, :], wv_d[128 * t:128 * (t + 1), :])
                nc.sync.dma_start(wo_sb[:, t, :], wo_d[128 * t:128 * (t + 1), :])
            bq_sb = wpool.tile([128, NT], f32, tag="bq")
            bk_sb = wpool.tile([128, NT], f32, tag="bk")
            nc.sync.dma_start(bq_sb[:, :], bq_d[:, :])
            nc.sync.dma_start(bk_sb[:, :], bk_d[:, :])
            ones_sb = wpool.tile([128, 1], bf16, tag="ones")
            nc.vector.memset(ones_sb[:, :], 1.0)
            ones_row = wpool.tile([1, 128], f32, tag="onesr")
            nc.vector.memset(ones_row[:, :], 1.0)

            pools = (kvpool, instream, qinp, qpool, ptp, otp, rpool, outp,
                     ps_proj, ps_s, ps_o)
            aps = (qt_d, kt_d, vt_d, out_d,
                   wq_sb, wk_sb, wv_sb, wo_sb, bq_sb, bk_sb, ones_sb,
                   ones_row)
            loop_cm = (tc.For_i(0, loop_k, 1) if loop_k > 1
                       else contextlib.nullcontext())
            with loop_cm:
                _body(nc, mybir, pools, aps)

            if dummy_d is not None:
                nc.sync.dma_start(dummy_d[:, :], bq_sb[:, :])

    nc.compile()
    return nc


def _get_compiled():
    global _compiled
    if _compiled is None:
        _compiled = _build()
    return _compiled


def _make_in_maps(Q, K, V, Wq, bq, Wk, bk, Wv, bv, Wo, bo):
    bf = ml_dtypes.bfloat16
    qt = np.ascontiguousarray(Q.reshape(BS, D).T).astype(bf)
    kt = np.ascontiguousarray(K.reshape(BS, D).T).astype(bf)
    vt = np.ascontiguousarray(V.reshape(BS, D).T).astype(bf)
    in_maps = []
    for h in range(N_CORES):
        sl = slice(D * h, D * (h + 1))
        in_maps.append({
            "qt": qt, "kt": kt, "vt": vt,
            "wq": np.ascontiguousarray(Wq[:, sl]).astype(bf),
            "wk": np.ascontiguousarray(Wk[:, sl]).astype(bf),
            "wv": np.ascontiguousarray(Wv[:, sl]).astype(bf),
            "wo": np.ascontiguousarray(Wo[sl, :]).astype(bf),
            "bq": np.ascontiguousarray(bq[sl].reshape(NT, 128).T).astype(np.float32),
            "bk": np.ascontiguousarray(bk[sl].reshape(NT, 128).T).astype(np.float32),
        })
    return in_maps


def kernel(Q, K, V, Wq, bq, Wk, bk, Wv, bv, Wo, bo, _trace=False, _trace_kwargs=None):
    from concourse.bass_utils import run_bass_kernel_spmd

    nc = _get_compiled()
    in_maps = _make_in_maps(Q, K, V, Wq, bq, Wk, bk, Wv, bv, Wo, bo)
    res = run_bass_kernel_spmd(nc, in_maps, core_ids=list(range(N_CORES)),
                               trace=_trace, **(_trace_kwargs or {}))
    kernel.last_results = res

    acc = np.zeros((BS, D), np.float64)
    for h in range(N_CORES):
        acc += res.results[h]["out"].astype(np.float64)
    const = bv.astype(np.float64) @ Wo.astype(np.float64) + bo.astype(np.float64)
    return (acc + const).astype(np.float32).reshape(B, S, D)


# revision 21
# speedup vs baseline: 1.6944x; 1.1344x over previous
"""Multi-head attention TRN2 kernel, head-parallel over 8 NeuronCores.

Problem shape: B=2, S=2048, d_model=512, n_heads=8, head_dim=512
(the projections are Linear(512, 512*8), so each head has dim 512).

Sharding: core h computes head h for both batches (column-parallel
Wq/Wk/Wv, row-parallel Wo).  Each core returns its partial output
(O_h @ Wo_h) of shape [B*S, 512]; the host sums the 8 partials and adds
the bias constant (bv @ Wo + bo), which passes through attention
linearly because softmax rows sum to 1.

Device layout avoids all on-device transposes:
  - Host passes Q/K/V pre-transposed as [512, B*S] bf16.
  - q,k are produced transposed ([head_dim, S]); v natural ([S, head_dim]).
  - Scores are computed transposed (P^T tiles [Sk, Sq]); exp on ACT
    (no max subtraction: |scores| <= ~2.5 for this problem's scale).
  - PV matmuls with v as stationary give O^T directly; softmax
    denominators come from a ones-vector matmul over the same P^T tiles;
    normalization is a DVE multiply against a partition-broadcast
    reciprocal row. O^T blocks are then the stationary operand for the
    Wo matmul.
"""

import math

import numpy as np
import ml_dtypes

B = 2
S = 2048
D = 512          # d_model == head_dim
H = 8
N_CORES = 8
BS = B * S       # 4096
NT = D // 128    # 4 contraction tiles of 128
SQC = 512        # query-chunk (matmul moving free dim)
NSQ = S // SQC   # 4 chunks per batch
NKT = S // 128   # 16 key tiles per batch
SCALE = 1.0 / math.sqrt(float(D))

_compiled = None


def _body(nc, mybir, pools, aps, pe_only=False):
    f32 = mybir.dt.float32
    bf16 = mybir.dt.bfloat16
    Id = mybir.ActivationFunctionType.Identity
    Exp = mybir.ActivationFunctionType.Exp

    class _Skip:
        def __getattr__(self, name):
            return lambda *a, **k: None

    act = _Skip() if pe_only else nc.scalar
    dve = _Skip() if pe_only else nc.vector
    (kvpool, instream, qinp, qpool, ptp, otp, rpool, outp,
     ps_proj, ps_s, ps_o) = pools
    (qt_d, kt_d, vt_d, out_d,
     wq_sb, wk_sb, wv_sb, wo_sb, bq_sb, bk_sb, ones_sb, ones_row) = aps

    bb = {}
    if pe_only:
        # static SBUF buffers standing in for tiles whose writers
        # (ACT/DVE) are skipped; PE reads garbage, timing-only build
        def _sb(name, shape, dt):
            return nc.alloc_sbuf_tensor(name, list(shape), dt).ap()
        bb["ktb"] = _sb("bb_ktb", [128, NT, S], bf16)
        bb["vb"] = _sb("bb_vb", [128, NKT, D], bf16)
        bb["qtc"] = _sb("bb_qtc", [128, NT, SQC], bf16)
        bb["otc"] = _sb("bb_otc", [128, NT, SQC], bf16)
        bb["rec"] = _sb("bb_rec", [1, SQC], f32)
        for j in range(NKT):
            bb["pt%d" % j] = _sb("bb_pt%d" % j, [128, SQC], bf16)

    for b in range(B):
        tok0 = S * b

        # --- k^T for this batch: kt_b[p, t, s] = k_h[128t+p, s] ---
        kin = []
        for t in range(NT):
            ki = instream.tile([128, S], bf16, tag="kin")
            nc.sync.dma_start(
                ki[:, :], kt_d[128 * t:128 * (t + 1), tok0:tok0 + S])
            kin.append(ki)
        kt_b = bb["ktb"] if pe_only else kvpool.tile([128, NT, S], bf16, tag="ktb")
        for m in range(NT):
            for c4 in range(NSQ):
                ps = ps_proj.tile([128, SQC], f32, tag="psp")
                for t in range(NT):
                    nc.tensor.matmul(
                        ps[:, :],
                        wk_sb[:, t, 128 * m:128 * (m + 1)],
                        kin[t][:, SQC * c4:SQC * (c4 + 1)],
                        start=(t == 0), stop=(t == NT - 1))
                act.activation(
                    kt_b[:, m, SQC * c4:SQC * (c4 + 1)], ps[:, :], Id,
                    bias=bk_sb[:, m:m + 1])

        # --- v natural for this batch: v_b[p, j, e] = v_h[128j+p, e] ---
        vin = []
        for t in range(NT):
            vi = instream.tile([128, S], bf16, tag="vin")
            nc.sync.dma_start(
                vi[:, :], vt_d[128 * t:128 * (t + 1), tok0:tok0 + S])
            vin.append(vi)
        v_b = bb["vb"] if pe_only else kvpool.tile([128, NKT, D], bf16, tag="vb")
        for j in range(NKT):
            ps = ps_proj.tile([128, D], f32, tag="psp")
            for t in range(NT):
                nc.tensor.matmul(
                    ps[:, :],
                    vin[t][:, 128 * j:128 * (j + 1)],
                    wv_sb[:, t, :],
                    start=(t == 0), stop=(t == NT - 1))
            act.activation(v_b[:, j, :], ps[:, :], Id)

        for c in range(NSQ):
            q0 = tok0 + SQC * c

            # --- q^T chunk: qt_c[p, m, s] = q_h[128m+p, q0+s] ---
            qin = []
            for t in range(NT):
                qi = qinp.tile([128, SQC], bf16, tag="qin")
                nc.sync.dma_start(
                    qi[:, :], qt_d[128 * t:128 * (t + 1), q0:q0 + SQC])
                qin.append(qi)
            qt_c = bb["qtc"] if pe_only else qpool.tile([128, NT, SQC], bf16, tag="qtc")
            for m in range(NT):
                ps = ps_proj.tile([128, SQC], f32, tag="psp")
                for t in range(NT):
                    nc.tensor.matmul(
                        ps[:, :],
                        wq_sb[:, t, 128 * m:128 * (m + 1)],
                        qin[t][:, :],
                        start=(t == 0), stop=(t == NT - 1))
                act.activation(
                    qt_c[:, m, :], ps[:, :], Id, bias=bq_sb[:, m:m + 1])

            # --- P^T tiles: pts[j][p, s] = exp(scores[q0+s, 128j+p]) ---
            pts = []
            for j in range(NKT):
                ps = ps_s.tile([128, SQC], f32, tag="pss")
                for t in range(NT):
                    nc.tensor.matmul(
                        ps[:, :],
                        kt_b[:, t, 128 * j:128 * (j + 1)],
                        qt_c[:, t, :],
                        start=(t == 0), stop=(t == NT - 1))
                pt_j = bb["pt%d" % j] if pe_only else ptp.tile([128, SQC], bf16, tag="ptj")
                act.activation(pt_j[:, :], ps[:, :], Exp, scale=SCALE)
                pts.append(pt_j)

            # --- softmax denominators, then O^T = v^T P^T ---
            pd = ps_o.tile([1, SQC], f32, tag="pso")
            for j in range(NKT):
                nc.tensor.matmul(pd[:, :], ones_sb[:, :], pts[j][:, :],
                                 start=(j == 0), stop=(j == NKT - 1))
            rec = bb["rec"] if pe_only else rpool.tile([1, SQC], f32, tag="rec")
            dve.reciprocal(rec[:, :], pd[:, :])
            # broadcast rec across partitions with a K=1 outer-product
            # matmul: [1,128] ones^T @ [1,512] rec -> psum [128,512],
            # then bounce to SBUF (DVE can read only one PSUM input)
            rec_ps = ps_s.tile([128, SQC], f32, tag="pss")
            nc.tensor.matmul(rec_ps[:, :], ones_row[:, :], rec[:, :],
                             start=True, stop=True)
            rec_full = None if pe_only else rpool.tile([128, SQC], f32, tag="recb")
            if not pe_only:
                act.copy(rec_full[:, :], rec_ps[:, :])

            ot_c = bb["otc"] if pe_only else otp.tile([128, NT, SQC], bf16, tag="otc")
            for m in range(NT):
                po = ps_o.tile([128, SQC], f32, tag="pso")
                for j in range(NKT):
                    nc.tensor.matmul(
                        po[:, :],
                        v_b[:, j, 128 * m:128 * (m + 1)],
                        pts[j][:, :],
                        start=(j == 0), stop=(j == NKT - 1))
                if not pe_only:
                    dve.tensor_mul(
                        ot_c[:, m, :], po[:, :], rec_full[:, :])

            # --- partial out chunk: O @ Wo_h ---
            for jq in range(NSQ):
                pw = ps_proj.tile([128, D], f32, tag="psp")
                for m in range(NT):
                    nc.tensor.matmul(
                        pw[:, :],
                        ot_c[:, m, 128 * jq:128 * (jq + 1)],
                        wo_sb[:, m, :],
                        start=(m == 0), stop=(m == NT - 1))
                if not pe_only:
                    osb = outp.tile([128, D], f32, tag="osb")
                    act.copy(osb[:, :], pw[:, :])
                    r0 = q0 + 128 * jq
                    nc.sync.dma_start(out_d[r0:r0 + 128, :], osb[:, :])


def _build(loop_k=1, bench=False, pe_only=False):
    """Build the bass program. loop_k>1 wraps the whole body in a
    For_i hardware loop; bench=True demotes the real output to an
    internal DRAM buffer and emits a tiny dummy ExternalOutput (both
    used only for differential timing in test.py; the graded path uses
    loop_k=1, bench=False)."""
    import contextlib

    import concourse.tile as tile
    from concourse import bacc, mybir

    f32 = mybir.dt.float32
    bf16 = mybir.dt.bfloat16

    nc = bacc.Bacc("TRN2", target_bir_lowering=False, debug=False,
                   num_devices=N_CORES)

    qt_d = nc.dram_tensor("qt", [D, BS], bf16, kind="ExternalInput").ap()
    kt_d = nc.dram_tensor("kt", [D, BS], bf16, kind="ExternalInput").ap()
    vt_d = nc.dram_tensor("vt", [D, BS], bf16, kind="ExternalInput").ap()
    wq_d = nc.dram_tensor("wq", [D, D], bf16, kind="ExternalInput").ap()
    wk_d = nc.dram_tensor("wk", [D, D], bf16, kind="ExternalInput").ap()
    wv_d = nc.dram_tensor("wv", [D, D], bf16, kind="ExternalInput").ap()
    wo_d = nc.dram_tensor("wo", [D, D], bf16, kind="ExternalInput").ap()
    bq_d = nc.dram_tensor("bq", [128, NT], f32, kind="ExternalInput").ap()
    bk_d = nc.dram_tensor("bk", [128, NT], f32, kind="ExternalInput").ap()
    if bench:
        out_d = nc.dram_tensor("outbuf", [BS, D], f32).ap()
        dummy_d = nc.dram_tensor("out", [128, NT], f32,
                                 kind="ExternalOutput").ap()
    else:
        out_d = nc.dram_tensor("out", [BS, D], f32, kind="ExternalOutput").ap()
        dummy_d = None

    with tile.TileContext(nc) as tc:
        with (
            tc.tile_pool(name="weights", bufs=1) as wpool,
            tc.tile_pool(name="kv", bufs=2) as kvpool,
            tc.tile_pool(name="instream", bufs=4) as instream,
            tc.tile_pool(name="qin", bufs=8) as qinp,
            tc.tile_pool(name="q", bufs=2) as qpool,
            tc.tile_pool(name="pt", bufs=20) as ptp,
            tc.tile_pool(name="ot", bufs=2) as otp,
            tc.tile_pool(name="recip", bufs=2) as rpool,
            tc.tile_pool(name="outsb", bufs=8) as outp,
            tc.tile_pool(name="ps_proj", bufs=2, space="PSUM") as ps_proj,
            tc.tile_pool(name="ps_s", bufs=3, space="PSUM") as ps_s,
            tc.tile_pool(name="ps_o", bufs=2, space="PSUM") as ps_o,
        ):
            # --- weights / biases, resident for the whole kernel ---
            wq_sb = wpool.tile([128, NT, D], bf16, tag="wq")
            wk_sb = wpool.tile([128, NT, D], bf16, tag="wk")
            wv_sb = wpool.tile([128, NT, D], bf16, tag="wv")
            wo_sb = wpool.tile([128, NT, D], bf16, tag="wo")
            for t in range(NT):
                nc.sync.dma_start(wq_sb[:, t, :], wq_d[128 * t:128 * (t + 1), :])
                nc.sync.dma_start(wk_sb[:, t, :], wk_d[128 * t:128 * (t + 1), :])
                nc.sync.dma_start(wv_sb[:, t, :], wv_d[128 * t:128 * (t + 1), :])
                nc.sync.dma_start(wo_sb[:, t, :], wo_d[128 * t:128 * (t + 1), :])
            bq_sb = wpool.tile([128, NT], f32, tag="bq")
            bk_sb = wpool.tile([128, NT], f32, tag="bk")
            nc.sync.dma_start(bq_sb[:, :], bq_d[:, :])
            nc.sync.dma_start(bk_sb[:, :], bk_d[:, :])
            ones_sb = wpool.tile([128, 1], bf16, tag="ones")
            nc.vector.memset(ones_sb[:, :], 1.0)
            ones_row = wpool.tile([1, 128], f32, tag="onesr")
            nc.vector.memset(ones_row[:, :], 1.0)

            pools = (kvpool, instream, qinp, qpool, ptp, otp, rpool, outp,
                     ps_proj, ps_s, ps_o)
            aps = (qt_d, kt_d, vt_d, out_d,
                   wq_sb, wk_sb, wv_sb, wo_sb, bq_sb, bk_sb, ones_sb,
                   ones_row)
            loop_cm = (tc.For_i(0, loop_k, 1) if loop_k > 1
                       else contextlib.nullcontext())
            with loop_cm:
                _body(nc, mybir, pools, aps, pe_only=pe_only)

            if dummy_d is not None:
                nc.sync.dma_start(dummy_d[:, :], bq_sb[:, :])

    nc.compile()
    return nc


def _get_compiled():
    global _compiled
    if _compiled is None:
        _compiled = _build()
    return _compiled


def _make_in_maps(Q, K, V, Wq, bq, Wk, bk, Wv, bv, Wo, bo):
    bf = ml_dtypes.bfloat16
    qt = np.ascontiguousarray(Q.reshape(BS, D).T).astype(bf)
    kt = np.ascontiguousarray(K.reshape(BS, D).T).astype(bf)
    vt = np.ascontiguousarray(V.reshape(BS, D).T).astype(bf)
    in_maps = []
    for h in range(N_CORES):
        sl = slice(D * h, D * (h + 1))
        in_maps.append({
            "qt": qt, "kt": kt, "vt": vt,
            "wq": np.ascontiguousarray(Wq[:, sl]).astype(bf),
            "wk": np.ascontiguousarray(Wk[:, sl]).astype(bf),
            "wv": np.ascontiguousarray(Wv[:, sl]).astype(bf),
            "wo": np.ascontiguousarray(Wo[sl, :]).astype(bf),
            "bq": np.ascontiguousarray(bq[sl].reshape(NT, 128).T).astype(np.float32),
            "bk": np.ascontiguousarray(bk[sl].reshape(NT, 128).T).astype(np.float32),
        })
    return in_maps


def kernel(Q, K, V, Wq, bq, Wk, bk, Wv, bv, Wo, bo, _trace=False, _trace_kwargs=None):
    from concourse.bass_utils import run_bass_kernel_spmd

    nc = _get_compiled()
    in_maps = _make_in_maps(Q, K, V, Wq, bq, Wk, bk, Wv, bv, Wo, bo)
    res = run_bass_kernel_spmd(nc, in_maps, core_ids=list(range(N_CORES)),
                               trace=_trace, **(_trace_kwargs or {}))
    kernel.last_results = res

    acc = np.zeros((BS, D), np.float64)
    for h in range(N_CORES):
        acc += res.results[h]["out"].astype(np.float64)
    const = bv.astype(np.float64) @ Wo.astype(np.float64) + bo.astype(np.float64)
    return (acc + const).astype(np.float32).reshape(B, S, D)
